# revision 25
# baseline (speedup 1.0000x reference)
"""Trainium2 Bass kernel for temporal-decay causal multi-head attention.

Problem: nn_MultiHeadAttention_9053791060340
  B=4, S=2048, DM=512, H=8, HD=64.
  out = softmax((Q K^T / sqrt(HD)) * exp(-rate*|t_i - t_j|) with causal mask) V,
  then out-projection.

Sharding: 8 cores = 4 batches x 2 head-groups (4 heads each). Each core
computes a partial out-projection [S, DM] for its head group; the host sums
the two partials per batch and adds the output bias.

Device algorithm (per core), matmul inputs bf16 with fp32 PSUM accumulation:
  - scores computed TRANSPOSED (S^T[k, q] = k . q); no-max softmax (scores
    bounded, exp never overflows fp32); denominator comes free from a
    ones-column appended to V (PV matmul accumulates ctx^T plus den row).
  - temporal decay factorizes on sorted days: exp(-r(t_q - t_k)) = a_q * b_k
    with a per-q-group reference t0 for fp32 range. a (with the 1/sqrt(HD)
    scale) is folded into a second host-prepared copy of x^T (xTa), so the
    q-projection emits pre-scaled q^T directly; b is folded into a per-group
    scaled k^T copy on gpsimd.
  - far-past pairs have weight exp(s*decay) within 1e-3 of 1.0 once
    rate*dist > ln(150/1e-3); whole k-chunks beyond that cutoff collapse to
    a rank-1 prefix update, applied per 128-q-block by initializing the PV
    PSUM accumulator with a K=4 f16 matmul (prefix V-sums + counts x block
    indicator). Streamed column ranges are also trimmed on the far side
    (q >= q_hi) at 128 granularity, halving score/exp/PV work vs a per-
    q-group cutoff.
  - causal masking only needs the diagonal 128x128 band: an additive -1e30
    tril tile applied to S^T in PSUM before the exp.
  - softmax denominators: DVE reciprocal directly on the PSUM den row, a
    K=1 fp32 matmul broadcasts 1/den across 64 partitions, and one DVE
    multiply rescales ctx (no PE transposes / partition_broadcast).
  - all DRAM parameters are host-pre-arranged so every DMA descriptor is a
    fat contiguous per-partition line (the naive layouts cost ~80ns per
    512B descriptor and dominated kernel startup).
"""

import os

import ml_dtypes
import numpy as np

import concourse.bass as bass
import concourse.tile as tile
from concourse import bacc
from concourse import mybir
from concourse.bass_utils import run_bass_kernel_spmd
from concourse.masks import make_identity

F32 = mybir.dt.float32
F16 = mybir.dt.float16
BF16 = mybir.dt.bfloat16

B, S, DM, H = 4, 2048, 512, 8
HD = DM // H          # 64
NCORES = 8
NHG = 2               # head groups == cores per batch
NH = H // NHG         # heads per core
HGD = NH * HD         # 256 output dims per core
QG = 512              # q-group width
NQG = S // QG         # 4
KC = 128              # k chunk (partition dim of S^T)
NKC = S // KC         # 16
NBLK = QG // KC       # 4 q-blocks per group
P = 128
KO = DM // P          # 4 contraction sub-chunks
VW = HD + 1           # 65: V columns plus ones column
NEG = -1.0e30
EPS = 1.0e-3          # max |exp(s*decay) - 1| treated as exactly 1

_cache: dict = {}



# --------------------------------------------------------------------------
# device program
# --------------------------------------------------------------------------

def _build_fast(bounds):
    """Build the SPMD Bass program.

    bounds = (kc_lo, qhi) with kc_lo[qg] = first streamed k-chunk of group
    qg, and qhi[qg][kc - kc_lo[qg]] = 128-rounded exclusive end of the
    streamed q-column range (group-relative) for that chunk. Static across
    cores (worst case over batches).
    """
    kc_lo, qhi = bounds
    wmax = max((qg + 1) * QG - kc_lo[qg] * KC for qg in range(NQG))
    maxc = max(
        qhi[qg][kc - kc_lo[qg]] - max(0, KC * (kc - 4 * qg))
        for qg in range(NQG) for kc in range(kc_lo[qg], 4 * qg + 4)
    )
    # PSUM budget (8 banks): ppool 2x1 + spool 2x2 + cpool 2x1. Matmul
    # outputs must start at a PSUM bank boundary, so the score pair tile is
    # always [P, 2, QG] (j slices bank-aligned) and ctx accumulators are
    # per-head single-bank tiles.
    assert maxc <= QG

    nc = bacc.Bacc()

    xa_d = nc.declare_dram_parameter("xTa", [P, NQG * KO * QG], BF16, False)
    xt_d = nc.declare_dram_parameter("xT2", [P, NQG * KO * QG], BF16, False)
    wq_d = nc.declare_dram_parameter("wqT", [P, KO * HGD], BF16, False)
    wk_d = nc.declare_dram_parameter("wkT", [P, KO * HGD], BF16, False)
    wv_d = nc.declare_dram_parameter("wvT", [P, KO * HGD], BF16, False)
    wo_d = nc.declare_dram_parameter("woT", [P, 2 * DM], BF16, False)
    bv_d = nc.declare_dram_parameter("bvec", [NQG, S], F32, False)
    pf_d = nc.declare_dram_parameter("pf4", [NBLK, NQG * NH * VW], F16, False)
    bk_d = nc.declare_dram_parameter("blk", [NBLK, QG], F16, False)
    bm_d = nc.declare_dram_parameter("bandm", [P, P], F32, False)
    out_d = nc.declare_dram_parameter("outp", [S, DM], F32, True)
    # DRAM scratch for the 1/den rows: SBUF->SBUF DMA cannot partition-
    # broadcast, so bounce through DRAM and broadcast on the way back
    rs_d = nc.dram_tensor("recscr", [NQG, NH, QG], F32, kind="Internal")

    with tile.TileContext(nc) as tc:
        with (
            tc.tile_pool(name="const", bufs=1) as const,
            tc.tile_pool(name="ppool", bufs=2, space="PSUM") as ppool,
            tc.tile_pool(name="spool", bufs=2, space="PSUM") as spool,
            tc.tile_pool(name="cpool", bufs=2, space="PSUM") as cpool,
            tc.tile_pool(name="ptp", bufs=3) as ptp,
            tc.tile_pool(name="ktsp", bufs=3) as ktsp,
            tc.tile_pool(name="bvqp", bufs=3) as bvqp,
            tc.tile_pool(name="ctxp", bufs=3) as ctxp,
            tc.tile_pool(name="csbp", bufs=8) as csbp,
            tc.tile_pool(name="recp", bufs=8) as recp,
            tc.tile_pool(name="cprp", bufs=4) as cprp,
            tc.tile_pool(name="osbp", bufs=3) as osbp,
        ):
            # ---- constant loads, first-needed first; every DMA below is
            # 128 (or fewer) contiguous per-partition descriptors ----
            wq_sb = const.tile([P, KO, HGD], BF16)
            nc.sync.dma_start(wq_sb, wq_d[:].rearrange("p (ko m) -> p ko m", m=HGD))
            xTa_sb = const.tile([P, NQG, KO, QG], BF16)
            xT_sb = const.tile([P, NQG, KO, QG], BF16)
            xa_r = xa_d[:].rearrange("p (ns ko q) -> p ns ko q", ko=KO, q=QG)
            xt_r = xt_d[:].rearrange("p (ns ko q) -> p ns ko q", ko=KO, q=QG)
            nc.sync.dma_start(xTa_sb[:, 0], xa_r[:, 0])
            wk_sb = const.tile([P, KO, HGD], BF16)
            nc.sync.dma_start(wk_sb, wk_d[:].rearrange("p (ko m) -> p ko m", m=HGD))
            nc.sync.dma_start(xT_sb[:, 0], xt_r[:, 0])
            wv_sb = const.tile([P, KO, HGD], BF16)
            nc.sync.dma_start(wv_sb, wv_d[:].rearrange("p (ko m) -> p ko m", m=HGD))
            pf_sb = const.tile([NBLK, NQG, NH * VW], F16)
            nc.sync.dma_start(
                pf_sb, pf_d[:].rearrange("r (g m) -> r g m", g=NQG))
            blk_sb = const.tile([NBLK, QG], F16)
            nc.sync.dma_start(blk_sb, bk_d[:])
            bm_sb = const.tile([P, P], F32)
            nc.sync.dma_start(bm_sb, bm_d[:])
            for ns in range(1, NQG):
                nc.sync.dma_start(xTa_sb[:, ns], xa_r[:, ns])
                nc.sync.dma_start(xT_sb[:, ns], xt_r[:, ns])
            wo_sb = const.tile([P, 2, DM], BF16)
            nc.sync.dma_start(wo_sb, wo_d[:].rearrange("p (hp n) -> p hp n", n=DM))

            eye4_sb = const.tile([NH, NH], F32)
            make_identity(nc, eye4_sb)
            eye128_sb = const.tile([P, P], F32)
            make_identity(nc, eye128_sb)

            qT_sb = const.tile([P, 2, S], BF16)
            kT_sb = const.tile([P, 2, S], F32)
            va_sb = const.tile([P, NKC, NH * VW], BF16)
            va_resh = va_sb.rearrange("p s (h c) -> p s h c", c=VW)
            nc.vector.memset(va_resh[:, :, :, HD], 1.0)

            # ---- projections (emitted per q-group, interleaved with the
            # attention stream so the PE stays dense) ----
            def proj(ns):
                sl = slice(ns * QG, (ns + 1) * QG)
                for x_src, w_sb, t_sb in (
                    (xTa_sb, wq_sb, qT_sb),
                    (xT_sb, wk_sb, kT_sb),
                ):
                    for mc in range(2):
                        ps = ppool.tile([P, QG], F32, tag="pp")
                        for ki in range(KO):
                            nc.tensor.matmul(
                                ps,
                                lhsT=w_sb[:, ki, mc * P:(mc + 1) * P],
                                rhs=x_src[:, ns, ki, :],
                                start=(ki == 0),
                                stop=(ki == KO - 1),
                            )
                        # elastic: keep these out of the exp stream's way in
                        # the ACT FIFO
                        with tc.high_priority(offset=-600):
                            nc.scalar.copy(t_sb[:, mc, sl], ps)
                for sc in range(4):
                    ps = ppool.tile([P, HGD], F32, tag="pp")
                    for ki in range(KO):
                        nc.tensor.matmul(
                            ps,
                            lhsT=xT_sb[:, ns, ki, sc * P:(sc + 1) * P],
                            rhs=wv_sb[:, ki, :],
                            start=(ki == 0),
                            stop=(ki == KO - 1),
                        )
                    ps_resh = ps.rearrange("p (h c) -> p h c", c=HD)
                    with tc.high_priority(offset=-600):
                        nc.vector.tensor_copy(
                            va_resh[:, 4 * ns + sc, :, :HD], ps_resh)

            def prep(qg):
                """b-vector broadcast DMA + b-scaled k^T for group qg."""
                klo = kc_lo[qg] * KC
                khi = (qg + 1) * QG
                kw = khi - klo
                bvf = bvqp.tile([P, wmax], F32, tag="bvf")
                nc.sync.dma_start(
                    bvf[:, :kw],
                    bv_d[:][qg:qg + 1, klo:khi].to_broadcast([P, kw]),
                )
                kts = ktsp.tile([P, 2, wmax], BF16, tag="kts")
                for mc in range(2):
                    nc.gpsimd.tensor_tensor(
                        kts[:, mc, :kw], kT_sb[:, mc, klo:khi], bvf[:, :kw],
                        mybir.AluOpType.mult,
                    )
                return kts

            def attn(qg, kts):
                """score/exp/PV chains for all heads; returns sbuf ctx pairs
                (undivided) and reciprocal rows."""
                klo = kc_lo[qg] * KC
                kcs = list(range(kc_lo[qg], 4 * (qg + 1)))
                cxf4 = ctxp.tile([VW, NH, QG], F32, tag="cxf")
                for hp in range(2):
                    h0 = 2 * hp
                    cps_pair = []
                    # distant-past prefix + counts, per 128-q-block: K=4
                    # f16 matmul seeds each accumulator (and its den row)
                    for j in range(2):
                        cps = cpool.tile([VW, QG], F32, tag="ctx")
                        cps_pair.append(cps)
                        nc.tensor.matmul(
                            cps,
                            lhsT=pf_sb[:, qg,
                                       (h0 + j) * VW:(h0 + j + 1) * VW],
                            rhs=blk_sb,
                            start=True,
                            stop=False,
                        )
                    for kc in kcs:
                        q_off = max(0, KC * (kc - 4 * qg))
                        q_hi = qhi[qg][kc - kc_lo[qg]]
                        cols = q_hi - q_off
                        co = kc * KC - klo
                        sp2 = spool.tile([P, 2, QG], F32, tag="spsum")
                        for j in range(2):
                            pb = j * HD
                            nc.tensor.matmul(
                                sp2[:, j, :cols],
                                lhsT=kts[pb:pb + HD, hp, co:co + KC],
                                rhs=qT_sb[pb:pb + HD, hp,
                                          qg * QG + q_off:qg * QG + q_hi],
                                start=True,
                                stop=True,
                            )
                        if kc >= 4 * qg:  # diagonal: mask both heads' bands
                            band = bass.AP(
                                tensor=sp2.tensor, offset=sp2.offset,
                                ap=[list(sp2.ap[0]), [QG, 2], [1, KC]],
                            )
                            nc.vector.tensor_tensor(
                                band, band,
                                bm_sb[:, None, :].to_broadcast([P, 2, KC]),
                                mybir.AluOpType.add,
                            )
                        pt = ptp.tile([P, 2, maxc], BF16, tag="pt")
                        nc.scalar.activation(
                            pt[:, :, :cols], sp2[:, :, :cols],
                            mybir.ActivationFunctionType.Exp,
                        )
                        for j in range(2):
                            nc.tensor.matmul(
                                cps_pair[j][:, q_off:q_hi],
                                lhsT=va_sb[:, kc,
                                           (h0 + j) * VW:(h0 + j + 1) * VW],
                                rhs=pt[:, j, :cols],
                                start=False,
                                stop=(kc == kcs[-1]),
                            )
                    # undivided ctx (and its den row, partition 64) to
                    # SBUF, freeing the accumulation banks; normal priority
                    # -- these gate the next group's accumulators and the
                    # reciprocal chain
                    for j in range(2):
                        nc.scalar.copy(cxf4[:, 2 * hp + j, :], cps_pair[j])
                return cxf4

            def recip(qg, cxf4):
                # gather the 4 den rows onto partitions 0-3, transpose to
                # [128, 16] for a cheap 16-wide DVE reciprocal, transpose
                # back to [4, 512] rec rows
                dens = recp.tile([NH, QG], F32, tag="dens")
                nc.sync.dma_start(dens, cxf4[HD:HD + 1, :, :])
                dtp = ppool.tile([P, NH * NBLK], F32, tag="pp")
                for ss in range(NBLK):
                    nc.tensor.matmul(
                        dtp[:, ss * NH:(ss + 1) * NH],
                        lhsT=dens[:, ss * P:(ss + 1) * P],
                        rhs=eye4_sb,
                        start=True,
                        stop=True,
                    )
                rct = recp.tile([P, NH * NBLK], F32, tag="rct")
                nc.vector.reciprocal(rct, dtp)
                rps = ppool.tile([NH, QG], F32, tag="pp")
                for ss in range(NBLK):
                    nc.tensor.matmul(
                        rps[:, ss * P:(ss + 1) * P],
                        lhsT=rct[:, ss * NH:(ss + 1) * NH],
                        rhs=eye128_sb,
                        start=True,
                        stop=True,
                    )
                rec = recp.tile([NH, QG], F32, tag="rec")
                nc.vector.tensor_copy(rec, rps)
                nc.sync.dma_start(rs_d[:][qg], rec)
                return rec

            def tail(qg, cxf4, rec):
                """broadcast 1/den rows across 64 partitions on the idle
                gpsimd, rescale ctx, repack head pairs, out-project, store."""
                pairs = []
                for hp in range(2):
                    cp2 = cprp.tile([P, QG], BF16, tag="cpair")
                    for j in range(2):
                        h = 2 * hp + j
                        bcsd = csbp.tile([HD, QG], F32, tag="bcsd")
                        nc.sync.dma_start(
                            bcsd,
                            rs_d[:][qg, h:h + 1, :].to_broadcast([HD, QG]))
                        csb = csbp.tile([HD, QG], BF16, tag="csb")
                        nc.vector.tensor_tensor(
                            csb, cxf4[:HD, h, :], bcsd,
                            mybir.AluOpType.mult,
                        )
                        nc.sync.dma_start(
                            cp2[j * HD:(j + 1) * HD, :], csb)
                    pairs.append(cp2)
                for ss in range(QG // P):
                    ops = ppool.tile([P, DM], F32, tag="pp")
                    for hp in range(2):
                        nc.tensor.matmul(
                            ops,
                            lhsT=pairs[hp][:, ss * P:(ss + 1) * P],
                            rhs=wo_sb[:, hp, :],
                            start=(hp == 0),
                            stop=(hp == 1),
                        )
                    osb = osbp.tile([P, DM], F32, tag="osb")
                    with tc.high_priority(offset=-600):
                        nc.vector.tensor_copy(osb, ops)
                    nc.sync.dma_start(
                        out_d[:][qg * QG + ss * P:qg * QG + (ss + 1) * P, :],
                        osb,
                    )

            # two-group lookahead: projections + scaled-k prefetch run well
            # ahead of the attention group that consumes them
            proj(0)
            ktss = [prep(0)]
            proj(1)
            ktss.append(prep(1))
            for qg in range(NQG):
                cxf4 = attn(qg, ktss[qg])
                if qg + 2 < NQG:
                    proj(qg + 2)
                    ktss.append(prep(qg + 2))
                rec = recip(qg, cxf4)
                tail(qg, cxf4, rec)

    nc.finalize()
    return nc


# --------------------------------------------------------------------------
# host wrapper
# --------------------------------------------------------------------------

def _is_tril(mask: np.ndarray) -> bool:
    tril = np.tril(np.ones((S, S), dtype=mask.dtype))
    return all(np.array_equal(mask[b], tril) for b in range(mask.shape[0]))


def _sbuf_major(a: np.ndarray) -> np.ndarray:
    """[KO*P, F] DRAM matrix -> [P, KO*F] partition-major layout."""
    ko = a.shape[0] // P
    return np.ascontiguousarray(
        a.reshape(ko, P, -1).transpose(1, 0, 2).reshape(P, -1))


def _x_major(a: np.ndarray) -> np.ndarray:
    """[KO*P, S] x^T matrix -> [P, NQG*KO*QG] with free dims (ns, ko, q),
    matching the device's per-q-group slice DMAs."""
    return np.ascontiguousarray(
        a.reshape(KO, P, NQG, QG).transpose(1, 2, 0, 3).reshape(P, -1))


def _bounds(t64: np.ndarray, rate: float):
    """Static streamed-window bounds, worst case over batches."""
    d_cut = (np.log(150.0) + np.log(1.0 / EPS)) / rate
    # qhi_static[kc] = first q (global) with t_q >= t_chunk_end + d_cut,
    # max over batches
    qhi_static = np.zeros(NKC, np.int64)
    for b in range(B):
        tend = t64[b, KC - 1::KC]  # [NKC]
        qhi_static = np.maximum(
            qhi_static, np.searchsorted(t64[b], tend + d_cut, side="left"))
    kc_lo, qhi, kpre = [], [], []
    for qg in range(NQG):
        qr = []
        lo = None
        for kc in range(4 * qg + 4):
            q_off = max(0, KC * (kc - 4 * qg))
            r = min(QG, -(-max(0, int(qhi_static[kc]) - qg * QG) // KC) * KC)
            if r > q_off and lo is None:
                lo = kc
            if lo is not None:
                qr.append(r)
        kc_lo.append(lo)
        qhi.append(tuple(qr))
        # kpre[qg][r] = first kc with qhi_r > 128*r (chunks below it are
        # covered by the prefix for q-block r)
        kp = []
        for r in range(NBLK):
            kc = 0
            while kc < 4 * qg + r and (
                    min(QG, -(-max(0, int(qhi_static[kc]) - qg * QG)
                              // KC) * KC) <= KC * r):
                kc += 1
            kp.append(kc)
        kpre.append(tuple(kp))
    return tuple(kc_lo), tuple(qhi), tuple(kpre)


def _prep_core_inputs(x, days, Wq, Wk, Wv, Wo, rate):
    t = days.astype(np.float64)  # [B, S]
    kc_lo, qhi, kpre = _bounds(t, rate)

    # per-batch decay factor vectors (f64 for exactness, then f32)
    scale = 1.0 / np.sqrt(HD)
    t0 = np.stack([(t[:, qg * QG] + t[:, qg * QG + QG - 1]) * 0.5
                   for qg in range(NQG)], axis=1)  # [B, NQG]
    avec = np.zeros((B, S), np.float64)
    bvec = np.zeros((B, NQG, S), np.float32)
    for b in range(B):
        for qg in range(NQG):
            sl = slice(qg * QG, (qg + 1) * QG)
            avec[b, sl] = np.exp(-rate * (t[b, sl] - t0[b, qg])) * scale
            hi = (qg + 1) * QG
            bvec[b, qg, :hi] = (np.exp(rate * (t[b, :hi] - t0[b, qg]))
                                ).astype(np.float32)
    assert np.all(np.isfinite(avec)) and np.all(np.isfinite(bvec)), \
        "decay factor overflow; q-group span too large for fast path"

    # band mask: keep (0.0) iff q_local >= k_local else -1e30
    kl = np.arange(P)[:, None]
    ql = np.arange(P)[None, :]
    bandm = np.where(ql >= kl, 0.0, NEG).astype(np.float32)
    # q-block indicator [NBLK, QG]
    blk = np.zeros((NBLK, QG), np.float16)
    for r in range(NBLK):
        blk[r, r * KC:(r + 1) * KC] = 1.0

    bf = np.dtype(ml_dtypes.bfloat16)
    in_maps = []
    for c in range(NCORES):
        b, hg = divmod(c, NHG)
        cols = slice(hg * HGD, (hg + 1) * HGD)
        # per-q-block prefix V sums + counts: [NBLK, NQG*NH*VW] f16
        csum = np.concatenate(
            [np.zeros((1, DM)),
             np.cumsum(x[b].astype(np.float64).reshape(NKC, KC, DM)
                       .sum(axis=1), axis=0)], axis=0)  # [NKC+1, DM]
        WvT = Wv[cols, :].astype(np.float64).T  # [DM, HGD]
        pf4 = np.zeros((NBLK, NQG, NH, VW), np.float64)
        for qg in range(NQG):
            for r in range(NBLK):
                kp = kpre[qg][r]
                if kp > 0:
                    vs = csum[kp] @ WvT  # [HGD]
                    pf4[r, qg, :, :HD] = vs.reshape(NH, HD)
                pf4[r, qg, :, HD] = float(kp * KC)
        xb = x[b].astype(np.float64)
        m = {
            "xT2": _x_major(np.ascontiguousarray(x[b].T)).astype(bf),
            "xTa": _x_major(
                np.ascontiguousarray((xb * avec[b][:, None]).T)).astype(bf),
            "wqT": _sbuf_major(
                np.ascontiguousarray(Wq[cols, :].T)).astype(bf),
            "wkT": _sbuf_major(
                np.ascontiguousarray(Wk[cols, :].T)).astype(bf),
            "wvT": _sbuf_major(
                np.ascontiguousarray(Wv[cols, :].T)).astype(bf),
            "woT": _sbuf_major(
                np.ascontiguousarray(Wo[:, cols].T)).astype(bf),
            "bvec": bvec[b],
            "pf4": np.ascontiguousarray(
                pf4.reshape(NBLK, NQG * NH * VW)).astype(np.float16),
            "blk": blk,
            "bandm": bandm,
        }
        in_maps.append(m)
    return in_maps, (kc_lo, qhi)


def _reference_host(x, mask, days_offset, Wq, bq, Wk, bk, Wv, bv, Wo, bo,
                    decay_rate):
    """Emergency numpy fallback for inputs outside the fast path."""
    b, s, _ = x.shape
    out = np.empty((b, s, DM), np.float32)
    for bi in range(b):
        q = (x[bi] @ Wq.T + bq).reshape(s, H, HD).transpose(1, 0, 2)
        k = (x[bi] @ Wk.T + bk).reshape(s, H, HD).transpose(1, 0, 2)
        v = (x[bi] @ Wv.T + bv).reshape(s, H, HD).transpose(1, 0, 2)
        dist = np.abs(days_offset[bi][:, None] - days_offset[bi][None, :])
        decay = np.exp(-decay_rate * dist).astype(np.float32)
        ctx = np.empty((H, s, HD), np.float32)
        for h in range(H):
            sc = (q[h] @ k[h].T) / np.sqrt(HD) * decay
            sc = np.where(mask[bi] == 0, -np.inf, sc)
            sc = sc - sc.max(axis=-1, keepdims=True)
            e = np.exp(sc)
            ctx[h] = (e / e.sum(axis=-1, keepdims=True)) @ v[h]
        out[bi] = ctx.transpose(1, 0, 2).reshape(s, DM) @ Wo.T + bo
    return out


def kernel(x, mask, days_offset, Wq, bq, Wk, bk, Wv, bv, Wo, bo, decay_rate,
           _trace=False):
    x = np.asarray(x, np.float32)
    mask = np.asarray(mask)
    days = np.asarray(days_offset, np.float32)
    Wq, bq = np.asarray(Wq, np.float32), np.asarray(bq, np.float32)
    Wk, bk = np.asarray(Wk, np.float32), np.asarray(bk, np.float32)
    Wv, bv = np.asarray(Wv, np.float32), np.asarray(bv, np.float32)
    Wo, bo = np.asarray(Wo, np.float32), np.asarray(bo, np.float32)
    rate = float(np.asarray(decay_rate))

    sorted_ok = bool(np.all(np.diff(days, axis=-1) >= 0))
    no_bias = not (np.any(bq != 0) or np.any(bk != 0) or np.any(bv != 0))
    if not (sorted_ok and no_bias and rate > 0 and _is_tril(mask)):
        return _reference_host(x, mask, days, Wq, bq, Wk, bk, Wv, bv, Wo, bo,
                               rate)

    in_maps, bounds = _prep_core_inputs(x, days, Wq, Wk, Wv, Wo, rate)

    if bounds not in _cache:
        _cache[bounds] = _build_fast(bounds)
    nc = _cache[bounds]

    res = run_bass_kernel_spmd(nc, in_maps, core_ids=list(range(NCORES)),
                               trace=_trace)
    out = np.empty((B, S, DM), np.float32)
    for b in range(B):
        out[b] = res.results[2 * b]["outp"] + res.results[2 * b + 1]["outp"] + bo
    if _trace:
        return out, res
    return out


# revision 26
# speedup vs baseline: 1.0902x; 1.0902x over previous
"""Trainium2 Bass kernel for temporal-decay causal multi-head attention.

Problem: nn_MultiHeadAttention_9053791060340
  B=4, S=2048, DM=512, H=8, HD=64.
  out = softmax((Q K^T / sqrt(HD)) * exp(-rate*|t_i - t_j|) with causal mask) V,
  then out-projection.

Sharding: 8 cores = 4 batches x 2 head-groups (4 heads each). Each core
computes a partial out-projection [S, DM] for its head group; the host sums
the two partials per batch and adds the output bias.

Device algorithm (per core), matmul inputs bf16 with fp32 PSUM accumulation:
  - scores computed TRANSPOSED (S^T[k, q] = k . q); no-max softmax (scores
    bounded, exp never overflows fp32); denominator comes free from a
    ones-column appended to V (PV matmul accumulates ctx^T plus den row).
  - temporal decay factorizes on sorted days: exp(-r(t_q - t_k)) = a_q * b_k
    with a per-q-group reference t0 for fp32 range. a (with the 1/sqrt(HD)
    scale) is folded into a second host-prepared copy of x^T (xTa), so the
    q-projection emits pre-scaled q^T directly; b is folded into a per-group
    scaled k^T copy on gpsimd.
  - far-past pairs have weight exp(s*decay) within 1e-3 of 1.0 once
    rate*dist > ln(150/1e-3); whole k-chunks beyond that cutoff collapse to
    a rank-1 prefix update, applied per 128-q-block by initializing the PV
    PSUM accumulator with a K=4 f16 matmul (prefix V-sums + counts x block
    indicator). Streamed column ranges are also trimmed on the far side
    (q >= q_hi) at 128 granularity, halving score/exp/PV work vs a per-
    q-group cutoff.
  - causal masking only needs the diagonal 128x128 band: an additive -1e30
    tril tile applied to S^T in PSUM before the exp.
  - softmax denominators: DVE reciprocal directly on the PSUM den row, a
    K=1 fp32 matmul broadcasts 1/den across 64 partitions, and one DVE
    multiply rescales ctx (no PE transposes / partition_broadcast).
  - all DRAM parameters are host-pre-arranged so every DMA descriptor is a
    fat contiguous per-partition line (the naive layouts cost ~80ns per
    512B descriptor and dominated kernel startup).
"""

import os

import ml_dtypes
import numpy as np

import concourse.bass as bass
import concourse.tile as tile
from concourse import bacc
from concourse import mybir
from concourse.bass_utils import run_bass_kernel_spmd
from concourse.masks import make_identity

F32 = mybir.dt.float32
F16 = mybir.dt.float16
BF16 = mybir.dt.bfloat16

B, S, DM, H = 4, 2048, 512, 8
HD = DM // H          # 64
NCORES = 8
NHG = 2               # head groups == cores per batch
NH = H // NHG         # heads per core
HGD = NH * HD         # 256 output dims per core
QG = 512              # q-group width
NQG = S // QG         # 4
KC = 128              # k chunk (partition dim of S^T)
NKC = S // KC         # 16
NBLK = QG // KC       # 4 q-blocks per group
P = 128
KO = DM // P          # 4 contraction sub-chunks
VW = HD + 1           # 65: V columns plus ones column
NEG = -1.0e30
EPS = 1.0e-3          # max |exp(s*decay) - 1| treated as exactly 1

_cache: dict = {}



# --------------------------------------------------------------------------
# device program
# --------------------------------------------------------------------------

def _build_fast(bounds):
    """Build the SPMD Bass program.

    bounds = (kc_lo, qhi) with kc_lo[qg] = first streamed k-chunk of group
    qg, and qhi[qg][kc - kc_lo[qg]] = 128-rounded exclusive end of the
    streamed q-column range (group-relative) for that chunk. Static across
    cores (worst case over batches).
    """
    kc_lo, qhi = bounds
    wmax = max((qg + 1) * QG - kc_lo[qg] * KC for qg in range(NQG))
    maxc = max(
        qhi[qg][kc - kc_lo[qg]] - max(0, KC * (kc - 4 * qg))
        for qg in range(NQG) for kc in range(kc_lo[qg], 4 * qg + 4)
    )
    # PSUM budget (8 banks): ppool 2x1 + spool 2x2 + cpool 2x1. Matmul
    # outputs must start at a PSUM bank boundary, so the score pair tile is
    # always [P, 2, QG] (j slices bank-aligned) and ctx accumulators are
    # per-head single-bank tiles.
    assert maxc <= QG

    nc = bacc.Bacc()

    xa_d = nc.declare_dram_parameter("xTa", [P, NQG * KO * QG], BF16, False)
    xt_d = nc.declare_dram_parameter("xT2", [P, NQG * KO * QG], BF16, False)
    wq_d = nc.declare_dram_parameter("wqT", [P, KO * HGD], BF16, False)
    wk_d = nc.declare_dram_parameter("wkT", [P, KO * HGD], BF16, False)
    wv_d = nc.declare_dram_parameter("wvT", [P, KO * HGD], BF16, False)
    wo_d = nc.declare_dram_parameter("woT", [P, 2 * DM], BF16, False)
    bv_d = nc.declare_dram_parameter("bvec", [NQG, S], F32, False)
    pf_d = nc.declare_dram_parameter("pf4", [NBLK, NQG * NH * VW], F16, False)
    bk_d = nc.declare_dram_parameter("blk", [NBLK, QG], F16, False)
    bm_d = nc.declare_dram_parameter("bandm", [P, P], F32, False)
    out_d = nc.declare_dram_parameter("outp", [S, DM], F32, True)
    # DRAM scratch for the 1/den rows: SBUF->SBUF DMA cannot partition-
    # broadcast, so bounce through DRAM and broadcast on the way back
    rs_d = nc.dram_tensor("recscr", [NQG, NH, QG], F32, kind="Internal")

    with tile.TileContext(nc) as tc:
        with (
            tc.tile_pool(name="const", bufs=1) as const,
            tc.tile_pool(name="ppool", bufs=2, space="PSUM") as ppool,
            tc.tile_pool(name="spool", bufs=2, space="PSUM") as spool,
            tc.tile_pool(name="cpool", bufs=2, space="PSUM") as cpool,
            tc.tile_pool(name="ptp", bufs=3) as ptp,
            tc.tile_pool(name="ktsp", bufs=3) as ktsp,
            tc.tile_pool(name="bvqp", bufs=3) as bvqp,
            tc.tile_pool(name="ctxp", bufs=3) as ctxp,
            tc.tile_pool(name="csbp", bufs=8) as csbp,
            tc.tile_pool(name="recp", bufs=8) as recp,
            tc.tile_pool(name="cprp", bufs=4) as cprp,
            tc.tile_pool(name="osbp", bufs=3) as osbp,
        ):
            # ---- constant loads, first-needed first; every DMA below is
            # 128 (or fewer) contiguous per-partition descriptors ----
            wq_sb = const.tile([P, KO, HGD], BF16)
            nc.sync.dma_start(wq_sb, wq_d[:].rearrange("p (ko m) -> p ko m", m=HGD))
            xTa_sb = const.tile([P, NQG, KO, QG], BF16)
            xT_sb = const.tile([P, NQG, KO, QG], BF16)
            xa_r = xa_d[:].rearrange("p (ns ko q) -> p ns ko q", ko=KO, q=QG)
            xt_r = xt_d[:].rearrange("p (ns ko q) -> p ns ko q", ko=KO, q=QG)
            nc.sync.dma_start(xTa_sb[:, 0], xa_r[:, 0])
            wk_sb = const.tile([P, KO, HGD], BF16)
            nc.sync.dma_start(wk_sb, wk_d[:].rearrange("p (ko m) -> p ko m", m=HGD))
            nc.sync.dma_start(xT_sb[:, 0], xt_r[:, 0])
            wv_sb = const.tile([P, KO, HGD], BF16)
            nc.sync.dma_start(wv_sb, wv_d[:].rearrange("p (ko m) -> p ko m", m=HGD))
            pf_sb = const.tile([NBLK, NQG, NH * VW], F16)
            nc.sync.dma_start(
                pf_sb, pf_d[:].rearrange("r (g m) -> r g m", g=NQG))
            blk_sb = const.tile([NBLK, QG], F16)
            nc.sync.dma_start(blk_sb, bk_d[:])
            bm_sb = const.tile([P, P], F32)
            nc.sync.dma_start(bm_sb, bm_d[:])
            for ns in range(1, NQG):
                nc.sync.dma_start(xTa_sb[:, ns], xa_r[:, ns])
                nc.sync.dma_start(xT_sb[:, ns], xt_r[:, ns])
            wo_sb = const.tile([P, 2, DM], BF16)
            nc.sync.dma_start(wo_sb, wo_d[:].rearrange("p (hp n) -> p hp n", n=DM))

            eye4_sb = const.tile([NH, NH], F32)
            make_identity(nc, eye4_sb)
            eye128_sb = const.tile([P, P], F32)
            make_identity(nc, eye128_sb)

            qT_sb = const.tile([P, 2, S], BF16)
            kT_sb = const.tile([P, 2, S], F32)
            va_sb = const.tile([P, NKC, NH * VW], BF16)
            va_resh = va_sb.rearrange("p s (h c) -> p s h c", c=VW)
            nc.vector.memset(va_resh[:, :, :, HD], 1.0)

            # ---- projections (emitted per q-group, interleaved with the
            # attention stream so the PE stays dense) ----
            def proj(ns):
                sl = slice(ns * QG, (ns + 1) * QG)
                for x_src, w_sb, t_sb in (
                    (xTa_sb, wq_sb, qT_sb),
                    (xT_sb, wk_sb, kT_sb),
                ):
                    for mc in range(2):
                        ps = ppool.tile([P, QG], F32, tag="pp")
                        for ki in range(KO):
                            nc.tensor.matmul(
                                ps,
                                lhsT=w_sb[:, ki, mc * P:(mc + 1) * P],
                                rhs=x_src[:, ns, ki, :],
                                start=(ki == 0),
                                stop=(ki == KO - 1),
                            )
                        # elastic: keep these out of the exp stream's way in
                        # the ACT FIFO
                        with tc.high_priority(offset=-600):
                            nc.scalar.copy(t_sb[:, mc, sl], ps)
                for sc in range(4):
                    ps = ppool.tile([P, HGD], F32, tag="pp")
                    for ki in range(KO):
                        nc.tensor.matmul(
                            ps,
                            lhsT=xT_sb[:, ns, ki, sc * P:(sc + 1) * P],
                            rhs=wv_sb[:, ki, :],
                            start=(ki == 0),
                            stop=(ki == KO - 1),
                        )
                    ps_resh = ps.rearrange("p (h c) -> p h c", c=HD)
                    with tc.high_priority(offset=-600):
                        nc.vector.tensor_copy(
                            va_resh[:, 4 * ns + sc, :, :HD], ps_resh)

            def prep(qg):
                """b-vector broadcast DMA + b-scaled k^T for group qg."""
                klo = kc_lo[qg] * KC
                khi = (qg + 1) * QG
                kw = khi - klo
                bvf = bvqp.tile([P, wmax], F32, tag="bvf")
                nc.sync.dma_start(
                    bvf[:, :kw],
                    bv_d[:][qg:qg + 1, klo:khi].to_broadcast([P, kw]),
                )
                kts = ktsp.tile([P, 2, wmax], BF16, tag="kts")
                for mc in range(2):
                    nc.gpsimd.tensor_tensor(
                        kts[:, mc, :kw], kT_sb[:, mc, klo:khi], bvf[:, :kw],
                        mybir.AluOpType.mult,
                    )
                return kts

            def attn(qg, kts):
                """score/exp/PV chains for all heads; returns sbuf ctx pairs
                (undivided) and reciprocal rows."""
                klo = kc_lo[qg] * KC
                kcs = list(range(kc_lo[qg], 4 * (qg + 1)))
                cxf4 = ctxp.tile([VW, NH, QG], F32, tag="cxf")
                for hp in range(2):
                    h0 = 2 * hp
                    cps_pair = []
                    # distant-past prefix + counts, per 128-q-block: K=4
                    # f16 matmul seeds each accumulator (and its den row)
                    for j in range(2):
                        cps = cpool.tile([VW, QG], F32, tag="ctx")
                        cps_pair.append(cps)
                        nc.tensor.matmul(
                            cps,
                            lhsT=pf_sb[:, qg,
                                       (h0 + j) * VW:(h0 + j + 1) * VW],
                            rhs=blk_sb,
                            start=True,
                            stop=False,
                        )
                    for kc in kcs:
                        q_off = max(0, KC * (kc - 4 * qg))
                        q_hi = qhi[qg][kc - kc_lo[qg]]
                        cols = q_hi - q_off
                        co = kc * KC - klo
                        sp2 = spool.tile([P, 2, QG], F32, tag="spsum")
                        for j in range(2):
                            pb = j * HD
                            nc.tensor.matmul(
                                sp2[:, j, :cols],
                                lhsT=kts[pb:pb + HD, hp, co:co + KC],
                                rhs=qT_sb[pb:pb + HD, hp,
                                          qg * QG + q_off:qg * QG + q_hi],
                                start=True,
                                stop=True,
                            )
                        if kc >= 4 * qg:  # diagonal: mask both heads' bands
                            band = bass.AP(
                                tensor=sp2.tensor, offset=sp2.offset,
                                ap=[list(sp2.ap[0]), [QG, 2], [1, KC]],
                            )
                            nc.vector.tensor_tensor(
                                band, band,
                                bm_sb[:, None, :].to_broadcast([P, 2, KC]),
                                mybir.AluOpType.add,
                            )
                        pt = ptp.tile([P, 2, maxc], BF16, tag="pt")
                        nc.scalar.activation(
                            pt[:, :, :cols], sp2[:, :, :cols],
                            mybir.ActivationFunctionType.Exp,
                        )
                        for j in range(2):
                            nc.tensor.matmul(
                                cps_pair[j][:, q_off:q_hi],
                                lhsT=va_sb[:, kc,
                                           (h0 + j) * VW:(h0 + j + 1) * VW],
                                rhs=pt[:, j, :cols],
                                start=False,
                                stop=(kc == kcs[-1]),
                            )
                    # undivided ctx (and its den row, partition 64) to
                    # SBUF, freeing the accumulation banks; normal priority
                    # -- these gate the next group's accumulators and the
                    # reciprocal chain
                    for j in range(2):
                        nc.scalar.copy(cxf4[:, 2 * hp + j, :], cps_pair[j])
                return cxf4

            def recip(qg, cxf4):
                # gather the 4 den rows onto partitions 0-3, transpose to
                # [128, 16] for a cheap 16-wide DVE reciprocal, transpose
                # back to [4, 512] rec rows
                dens = recp.tile([NH, QG], F32, tag="dens")
                nc.sync.dma_start(dens, cxf4[HD:HD + 1, :, :])
                dtp = ppool.tile([P, NH * NBLK], F32, tag="pp")
                for ss in range(NBLK):
                    nc.tensor.matmul(
                        dtp[:, ss * NH:(ss + 1) * NH],
                        lhsT=dens[:, ss * P:(ss + 1) * P],
                        rhs=eye4_sb,
                        start=True,
                        stop=True,
                    )
                rct = recp.tile([P, NH * NBLK], F32, tag="rct")
                nc.vector.reciprocal(rct, dtp)
                rps = ppool.tile([NH, QG], F32, tag="pp")
                for ss in range(NBLK):
                    nc.tensor.matmul(
                        rps[:, ss * P:(ss + 1) * P],
                        lhsT=rct[:, ss * NH:(ss + 1) * NH],
                        rhs=eye128_sb,
                        start=True,
                        stop=True,
                    )
                rec = recp.tile([NH, QG], F32, tag="rec")
                nc.vector.tensor_copy(rec, rps)
                nc.sync.dma_start(rs_d[:][qg], rec)
                return rec

            def tail(qg, cxf4, rec):
                """broadcast 1/den rows across 64 partitions on the idle
                gpsimd, rescale ctx, repack head pairs, out-project, store."""
                pairs = []
                for hp in range(2):
                    cp2 = cprp.tile([P, QG], BF16, tag="cpair")
                    for j in range(2):
                        h = 2 * hp + j
                        bcsd = csbp.tile([HD, QG], F32, tag="bcsd")
                        nc.sync.dma_start(
                            bcsd,
                            rs_d[:][qg, h:h + 1, :].to_broadcast([HD, QG]))
                        csb = csbp.tile([HD, QG], BF16, tag="csb")
                        nc.vector.tensor_tensor(
                            csb, cxf4[:HD, h, :], bcsd,
                            mybir.AluOpType.mult,
                        )
                        nc.sync.dma_start(
                            cp2[j * HD:(j + 1) * HD, :], csb)
                    pairs.append(cp2)
                for ss in range(QG // P):
                    ops = ppool.tile([P, DM], F32, tag="pp")
                    for hp in range(2):
                        nc.tensor.matmul(
                            ops,
                            lhsT=pairs[hp][:, ss * P:(ss + 1) * P],
                            rhs=wo_sb[:, hp, :],
                            start=(hp == 0),
                            stop=(hp == 1),
                        )
                    osb = osbp.tile([P, DM], F32, tag="osb")
                    with tc.high_priority(offset=-600):
                        nc.vector.tensor_copy(osb, ops)
                    nc.sync.dma_start(
                        out_d[:][qg * QG + ss * P:qg * QG + (ss + 1) * P, :],
                        osb,
                    )

            # two-group lookahead: projections + scaled-k prefetch run well
            # ahead of the attention group that consumes them
            proj(0)
            ktss = [prep(0)]
            proj(1)
            ktss.append(prep(1))
            pending = None
            for qg in range(NQG):
                cxf4 = attn(qg, ktss[qg])
                if qg + 2 < NQG:
                    proj(qg + 2)
                    ktss.append(prep(qg + 2))
                rec = recip(qg, cxf4)
                if pending is not None:
                    tail(*pending)
                pending = (qg, cxf4, rec)
            tail(*pending)

    nc.finalize()
    return nc


# --------------------------------------------------------------------------
# host wrapper
# --------------------------------------------------------------------------

def _is_tril(mask: np.ndarray) -> bool:
    tril = np.tril(np.ones((S, S), dtype=mask.dtype))
    return all(np.array_equal(mask[b], tril) for b in range(mask.shape[0]))


def _sbuf_major(a: np.ndarray) -> np.ndarray:
    """[KO*P, F] DRAM matrix -> [P, KO*F] partition-major layout."""
    ko = a.shape[0] // P
    return np.ascontiguousarray(
        a.reshape(ko, P, -1).transpose(1, 0, 2).reshape(P, -1))


def _x_major(a: np.ndarray) -> np.ndarray:
    """[KO*P, S] x^T matrix -> [P, NQG*KO*QG] with free dims (ns, ko, q),
    matching the device's per-q-group slice DMAs."""
    return np.ascontiguousarray(
        a.reshape(KO, P, NQG, QG).transpose(1, 2, 0, 3).reshape(P, -1))


def _bounds(t64: np.ndarray, rate: float):
    """Static streamed-window bounds, worst case over batches."""
    d_cut = (np.log(150.0) + np.log(1.0 / EPS)) / rate
    # qhi_static[kc] = first q (global) with t_q >= t_chunk_end + d_cut,
    # max over batches
    qhi_static = np.zeros(NKC, np.int64)
    for b in range(B):
        tend = t64[b, KC - 1::KC]  # [NKC]
        qhi_static = np.maximum(
            qhi_static, np.searchsorted(t64[b], tend + d_cut, side="left"))
    kc_lo, qhi, kpre = [], [], []
    for qg in range(NQG):
        qr = []
        lo = None
        for kc in range(4 * qg + 4):
            q_off = max(0, KC * (kc - 4 * qg))
            r = min(QG, -(-max(0, int(qhi_static[kc]) - qg * QG) // KC) * KC)
            if r > q_off and lo is None:
                lo = kc
            if lo is not None:
                qr.append(r)
        kc_lo.append(lo)
        qhi.append(tuple(qr))
        # kpre[qg][r] = first kc with qhi_r > 128*r (chunks below it are
        # covered by the prefix for q-block r)
        kp = []
        for r in range(NBLK):
            kc = 0
            while kc < 4 * qg + r and (
                    min(QG, -(-max(0, int(qhi_static[kc]) - qg * QG)
                              // KC) * KC) <= KC * r):
                kc += 1
            kp.append(kc)
        kpre.append(tuple(kp))
    return tuple(kc_lo), tuple(qhi), tuple(kpre)


def _prep_core_inputs(x, days, Wq, Wk, Wv, Wo, rate):
    t = days.astype(np.float64)  # [B, S]
    kc_lo, qhi, kpre = _bounds(t, rate)

    # per-batch decay factor vectors (f64 for exactness, then f32)
    scale = 1.0 / np.sqrt(HD)
    t0 = np.stack([(t[:, qg * QG] + t[:, qg * QG + QG - 1]) * 0.5
                   for qg in range(NQG)], axis=1)  # [B, NQG]
    avec = np.zeros((B, S), np.float64)
    bvec = np.zeros((B, NQG, S), np.float32)
    for b in range(B):
        for qg in range(NQG):
            sl = slice(qg * QG, (qg + 1) * QG)
            avec[b, sl] = np.exp(-rate * (t[b, sl] - t0[b, qg])) * scale
            hi = (qg + 1) * QG
            bvec[b, qg, :hi] = (np.exp(rate * (t[b, :hi] - t0[b, qg]))
                                ).astype(np.float32)
    assert np.all(np.isfinite(avec)) and np.all(np.isfinite(bvec)), \
        "decay factor overflow; q-group span too large for fast path"

    # band mask: keep (0.0) iff q_local >= k_local else -1e30
    kl = np.arange(P)[:, None]
    ql = np.arange(P)[None, :]
    bandm = np.where(ql >= kl, 0.0, NEG).astype(np.float32)
    # q-block indicator [NBLK, QG]
    blk = np.zeros((NBLK, QG), np.float16)
    for r in range(NBLK):
        blk[r, r * KC:(r + 1) * KC] = 1.0

    bf = np.dtype(ml_dtypes.bfloat16)
    in_maps = []
    for c in range(NCORES):
        b, hg = divmod(c, NHG)
        cols = slice(hg * HGD, (hg + 1) * HGD)
        # per-q-block prefix V sums + counts: [NBLK, NQG*NH*VW] f16
        csum = np.concatenate(
            [np.zeros((1, DM)),
             np.cumsum(x[b].astype(np.float64).reshape(NKC, KC, DM)
                       .sum(axis=1), axis=0)], axis=0)  # [NKC+1, DM]
        WvT = Wv[cols, :].astype(np.float64).T  # [DM, HGD]
        pf4 = np.zeros((NBLK, NQG, NH, VW), np.float64)
        for qg in range(NQG):
            for r in range(NBLK):
                kp = kpre[qg][r]
                if kp > 0:
                    vs = csum[kp] @ WvT  # [HGD]
                    pf4[r, qg, :, :HD] = vs.reshape(NH, HD)
                pf4[r, qg, :, HD] = float(kp * KC)
        xb = x[b].astype(np.float64)
        m = {
            "xT2": _x_major(np.ascontiguousarray(x[b].T)).astype(bf),
            "xTa": _x_major(
                np.ascontiguousarray((xb * avec[b][:, None]).T)).astype(bf),
            "wqT": _sbuf_major(
                np.ascontiguousarray(Wq[cols, :].T)).astype(bf),
            "wkT": _sbuf_major(
                np.ascontiguousarray(Wk[cols, :].T)).astype(bf),
            "wvT": _sbuf_major(
                np.ascontiguousarray(Wv[cols, :].T)).astype(bf),
            "woT": _sbuf_major(
                np.ascontiguousarray(Wo[:, cols].T)).astype(bf),
            "bvec": bvec[b],
            "pf4": np.ascontiguousarray(
                pf4.reshape(NBLK, NQG * NH * VW)).astype(np.float16),
            "blk": blk,
            "bandm": bandm,
        }
        in_maps.append(m)
    return in_maps, (kc_lo, qhi)


def _reference_host(x, mask, days_offset, Wq, bq, Wk, bk, Wv, bv, Wo, bo,
                    decay_rate):
    """Emergency numpy fallback for inputs outside the fast path."""
    b, s, _ = x.shape
    out = np.empty((b, s, DM), np.float32)
    for bi in range(b):
        q = (x[bi] @ Wq.T + bq).reshape(s, H, HD).transpose(1, 0, 2)
        k = (x[bi] @ Wk.T + bk).reshape(s, H, HD).transpose(1, 0, 2)
        v = (x[bi] @ Wv.T + bv).reshape(s, H, HD).transpose(1, 0, 2)
        dist = np.abs(days_offset[bi][:, None] - days_offset[bi][None, :])
        decay = np.exp(-decay_rate * dist).astype(np.float32)
        ctx = np.empty((H, s, HD), np.float32)
        for h in range(H):
            sc = (q[h] @ k[h].T) / np.sqrt(HD) * decay
            sc = np.where(mask[bi] == 0, -np.inf, sc)
            sc = sc - sc.max(axis=-1, keepdims=True)
            e = np.exp(sc)
            ctx[h] = (e / e.sum(axis=-1, keepdims=True)) @ v[h]
        out[bi] = ctx.transpose(1, 0, 2).reshape(s, DM) @ Wo.T + bo
    return out


def kernel(x, mask, days_offset, Wq, bq, Wk, bk, Wv, bv, Wo, bo, decay_rate,
           _trace=False):
    x = np.asarray(x, np.float32)
    mask = np.asarray(mask)
    days = np.asarray(days_offset, np.float32)
    Wq, bq = np.asarray(Wq, np.float32), np.asarray(bq, np.float32)
    Wk, bk = np.asarray(Wk, np.float32), np.asarray(bk, np.float32)
    Wv, bv = np.asarray(Wv, np.float32), np.asarray(bv, np.float32)
    Wo, bo = np.asarray(Wo, np.float32), np.asarray(bo, np.float32)
    rate = float(np.asarray(decay_rate))

    sorted_ok = bool(np.all(np.diff(days, axis=-1) >= 0))
    no_bias = not (np.any(bq != 0) or np.any(bk != 0) or np.any(bv != 0))
    if not (sorted_ok and no_bias and rate > 0 and _is_tril(mask)):
        return _reference_host(x, mask, days, Wq, bq, Wk, bk, Wv, bv, Wo, bo,
                               rate)

    in_maps, bounds = _prep_core_inputs(x, days, Wq, Wk, Wv, Wo, rate)

    if bounds not in _cache:
        _cache[bounds] = _build_fast(bounds)
    nc = _cache[bounds]

    res = run_bass_kernel_spmd(nc, in_maps, core_ids=list(range(NCORES)),
                               trace=_trace)
    out = np.empty((B, S, DM), np.float32)
    for b in range(B):
        out[b] = res.results[2 * b]["outp"] + res.results[2 * b + 1]["outp"] + bo
    if _trace:
        return out, res
    return out
